# revision 19
# baseline (speedup 1.0000x reference)
"""Causal linear attention (elu+1 feature map) Trainium2 Bass kernel.

Problem: queries/keys/values [N=4, L=8192, H=8, D=64] f32.
  Q = elu(q)+1, K = elu(k)+1
  Z = 1 / (sum_d Q[l,d] * cumsum_l(K)[l,d] + eps)
  out = causal_linear(Q, K, V) * Z          (chunked scan, CHUNK=128)

Sharding: 8 cores = (batch n in 0..3) x (head half in 0..1); each core
processes its [L, 4, 64] shard with no cross-core communication.

Per-core algorithm (per head h, chunk c of 128 rows):
  qT,kT = per-head PE transposes of the feature-mapped inputs, all at
          partitions 0-63 (uniform K=64 row group: consecutive matmuls
          with *different* partial row groups crash the PE here)
  attnT[s,t] = matmul(lhsT=kT_h, rhs=qT_h), masked by triu(s<=t)
  out[t,:]   = matmul(lhsT=attn_m, rhs=v'_h) + matmul(lhsT=qT_h, rhs=S'_h)
  v'    = v with a ones column appended, so the same matmuls also produce
          the normalizer denominator (rowsum(attn) + q . ksum)
  S'    = [S | ksum] PSUM accumulator over chunks (f32, exact)
  out_final = out * 1/denom via one broadcast tensor_tensor per chunk

Operands are fp16 (PE fast-weight-load + DVE 16-bit modes); all matmul
accumulation is f32 in PSUM; output f32.
"""

import numpy as np

import concourse.bass as bass_mod
import concourse.bacc as bacc
import concourse.tile as tile
from concourse import mybir
from concourse.bass_utils import run_bass_kernel_spmd

F16 = mybir.dt.float16
F32 = mybir.dt.float32
ALU = mybir.AluOpType
AF = mybir.ActivationFunctionType

N, L, H, D = 4, 8192, 8, 64
HL = 4                # heads per core
CHUNK = 128
NCHUNK = L // CHUNK   # 64
ITER_CHUNKS = 8       # chunks per outer iteration
NITER = NCHUNK // ITER_CHUNKS
CW = HL * D           # 256 floats per row in the shard

_CACHE = {}


def _build_body(nc, tc, q_d, k_d, v_d, msk_d, idn_d, out_d):
    qv = q_d.ap().rearrange("(i c p) h d -> i p c (h d)", p=CHUNK, c=ITER_CHUNKS)
    kv = k_d.ap().rearrange("(i c p) h d -> i p c (h d)", p=CHUNK, c=ITER_CHUNKS)
    vv = v_d.ap().rearrange("(i c p) h d -> i p c (h d)", p=CHUNK, c=ITER_CHUNKS)
    ov = out_d.ap().rearrange("(i c p) h d -> i p c (h d)", p=CHUNK, c=ITER_CHUNKS)
    IW = ITER_CHUNKS * CW        # 1024
    E1 = D + 1                   # 65: value cols + ones col
    VW = ITER_CHUNKS * HL * E1   # 1040: v' tile width

    def i3(t, lo, width):
        return t[:, lo : lo + width].rearrange("p (c w) -> p c w", w=CW)

    from contextlib import ExitStack
    ctx = ExitStack()
    consts = ctx.enter_context(tc.tile_pool(name="consts", bufs=1))
    mask_sb = consts.tile([128, HL * 128], F16)
    nc.sync.dma_start(out=mask_sb[:], in_=msk_d.ap())
    iden_sb = consts.tile([128, 128], F16)
    nc.sync.dma_start(out=iden_sb[:], in_=idn_d.ap())
    zero_sb = consts.tile([128, HL * E1], F16)
    nc.vector.memset(zero_sb[:], 0.0)
    ones_sb = consts.tile([128, 1], F16)
    nc.vector.memset(ones_sb[:], 1.0)

    qk_pool = ctx.enter_context(tc.tile_pool(name="qk", bufs=2))
    fm_pool = ctx.enter_context(tc.tile_pool(name="fm", bufs=2))
    v_pool = ctx.enter_context(tc.tile_pool(name="v", bufs=2))
    qkt_pool = ctx.enter_context(tc.tile_pool(name="qkt", bufs=3))
    am_pool = ctx.enter_context(tc.tile_pool(name="am", bufs=4))
    ssb_pool = ctx.enter_context(tc.tile_pool(name="ssb", bufs=3))
    rec_pool = ctx.enter_context(tc.tile_pool(name="rec", bufs=3))
    osb_pool = ctx.enter_context(tc.tile_pool(name="osb", bufs=2))

    ps_tp = ctx.enter_context(tc.tile_pool(name="ps_tp", bufs=2, space="PSUM"))
    ps_at = ctx.enter_context(tc.tile_pool(name="ps_at", bufs=3, space="PSUM"))
    ps_out = ctx.enter_context(tc.tile_pool(name="ps_out", bufs=2, space="PSUM"))
    ps_S = ctx.enter_context(tc.tile_pool(name="ps_S", bufs=1, space="PSUM"))

    # persistent state accumulator, all heads at partitions 0-63:
    # head h -> cols h*65..h*65+64 = S[d,e], col h*65+64 = ksum[d].
    # Opened by the first update (start=True); closed after chunk 0 by a
    # zero-accumulate matmul so later PSUM reads are legal; all later
    # updates accumulate with the group closed (skip_group_check).
    S_ps = ps_S.tile([64, HL * E1], F32)

    def load_iter(it):
        qk_t = qk_pool.tile([128, 2 * IW], F16)
        nc.gpsimd.dma_start(out=i3(qk_t, 0, IW), in_=qv[it])
        nc.gpsimd.dma_start(out=i3(qk_t, IW, IW), in_=kv[it])
        # v in straight (c h d) layout: one cast-DMA per iteration; the
        # denominator ones-column is handled by explicit 1-col matmuls
        v_t = v_pool.tile([128, IW], F16)
        nc.gpsimd.dma_start(out=i3(v_t, 0, IW), in_=vv[it])

        # feature map: elu(x)+1 == min(exp(x), relu(x)+1)
        e_t = fm_pool.tile([128, 2 * IW], F16, tag="fm_e")
        nc.scalar.activation(e_t[:], qk_t[:], AF.Exp)
        r_t = fm_pool.tile([128, 2 * IW], F16, tag="fm_r")
        nc.vector.tensor_scalar(r_t[:], qk_t[:], 0.0, 1.0, ALU.max, ALU.add)
        f_t = fm_pool.tile([128, 2 * IW], F16, tag="fm_f")
        nc.vector.tensor_tensor(f_t[:], e_t[:], r_t[:], ALU.min)

        out_sb = osb_pool.tile([128, IW], F16)
        return f_t, v_t, out_sb

    def phase_a1(g, f_t):
        """transposes + qkT PSUM->SBUF copies for chunk g (no S dependency)"""
        c = g % ITER_CHUNKS
        fq = f_t[:, c * CW : (c + 1) * CW]
        fk = f_t[:, IW + c * CW : IW + (c + 1) * CW]
        tp = ps_tp.tile([64, 8 * 128], F16)
        for h in range(HL):
            nc.tensor.transpose(
                tp[:, h * 128 : (h + 1) * 128],
                fq[:, h * D : (h + 1) * D], iden_sb[:])
            nc.tensor.transpose(
                tp[:, 512 + h * 128 : 512 + (h + 1) * 128],
                fk[:, h * D : (h + 1) * D], iden_sb[:])
        qkT = qkt_pool.tile([64, 8 * 128], F16)
        nc.scalar.copy(qkT[:, 0:512], tp[:, 0:512])
        nc.vector.tensor_copy(qkT[:, 512:1024], tp[:, 512:1024])
        return qkT, fk

    def phase_a2(qkT):
        """attn matmuls + mask for chunk g"""
        at_ps = ps_at.tile([128, HL * 128], F32)
        for h in range(HL):
            nc.tensor.matmul(
                at_ps[:, h * 128 : (h + 1) * 128],
                lhsT=qkT[:, 512 + h * 128 : 512 + (h + 1) * 128],
                rhs=qkT[:, h * 128 : (h + 1) * 128],
                start=(h == 0), stop=(h == HL - 1),
            )
        am = am_pool.tile([128, HL * 128], F16)
        nc.vector.tensor_tensor(am[:], at_ps[:], mask_sb[:], ALU.mult)
        return am

    def phase_b(g, am, qkT, fk, v_t, out_sb):
        """S copy, out matmuls, normalize, state update for chunk g"""
        c = g % ITER_CHUNKS
        S_sb = None
        if g > 0:
            S_sb = ssb_pool.tile([64, HL * E1], F16)
            nc.scalar.copy(S_sb[:], S_ps[:])

        # per head: am@v (values), am@ones (denom rowsum), q@S' (recurrent)
        out_ps = ps_out.tile([128, HL * E1], F32)
        n_mm = HL * (2 if g == 0 else 3)
        mm_i = 0
        for h in range(HL):
            co = h * E1
            amh = am[:, h * 128 : (h + 1) * 128]
            nc.tensor.matmul(
                out_ps[:, co : co + D],
                lhsT=amh,
                rhs=v_t[:, (c * HL + h) * D : (c * HL + h + 1) * D],
                start=(mm_i == 0), stop=(mm_i == n_mm - 1),
            )
            mm_i += 1
            nc.tensor.matmul(
                out_ps[:, co + D : co + E1],
                lhsT=amh,
                rhs=ones_sb[:],
                start=False, stop=(mm_i == n_mm - 1),
            )
            mm_i += 1
            if g > 0:
                nc.tensor.matmul(
                    out_ps[:, co : co + E1],
                    lhsT=qkT[:, h * 128 : (h + 1) * 128],
                    rhs=S_sb[:, co : co + E1],
                    start=False, stop=(mm_i == n_mm - 1),
                )
                mm_i += 1

        # ---- normalize: out = out_ps[:, value cols] * (1/denom)
        rec = rec_pool.tile([128, HL], F32)
        opv = out_ps[:].rearrange("p (h e) -> p h e", h=HL)
        nc.vector.reciprocal(rec[:], opv[:, :, D])
        rbc = bass_mod.AP(tensor=rec.tensor, offset=rec[:].offset,
                          ap=[list(rec[:].ap[0]), [1, HL], [0, D]])
        nc.vector.tensor_tensor(
            out_sb[:, c * CW : (c + 1) * CW].rearrange(
                "p (h e) -> p h e", h=HL),
            opv[:, :, 0:D], rbc, ALU.mult)

        # ---- state update (skip after last chunk)
        if g < NCHUNK - 1:
            for h in range(HL):
                fkh = fk[:, h * D : (h + 1) * D]
                nc.tensor.matmul(
                    S_ps[:, h * E1 : h * E1 + D],
                    lhsT=fkh,
                    rhs=v_t[:, (c * HL + h) * D : (c * HL + h + 1) * D],
                    start=(g == 0 and h == 0), stop=False,
                    skip_group_check=(g > 0),
                )
                nc.tensor.matmul(
                    S_ps[:, h * E1 + D : h * E1 + E1],
                    lhsT=fkh,
                    rhs=ones_sb[:],
                    start=False, stop=False,
                    skip_group_check=(g > 0),
                )
            if g == 0:
                # close the accumulation group (adds zeros) so later
                # engine reads of S_ps pass the sim's group check
                nc.tensor.matmul(
                    S_ps[:], lhsT=iden_sb[:, 0:64], rhs=zero_sb[:, 0 : HL * E1],
                    start=False, stop=True)

        if c == ITER_CHUNKS - 1:
            it = g // ITER_CHUNKS
            nc.gpsimd.dma_start(
                out=ov[it], in_=out_sb[:].rearrange("p (c w) -> p c w", w=CW))

    # software-pipelined PE issue order per step:
    #   [transposes(g)] [out/state(g-1)] [attn(g)]
    # so the Act qkT-copy of chunk g overlaps out/state of g-1, and the DVE
    # mask of chunk g overlaps the transposes of g+1
    pend = None
    iters = {}
    for g in range(NCHUNK):
        it, c = divmod(g, ITER_CHUNKS)
        if c == 0:
            iters[it] = load_iter(it)
        f_t, v_t, out_sb = iters[it]
        qkT, fk = phase_a1(g, f_t)
        am = phase_a2(qkT)
        phase_b(g, am, qkT, fk, v_t, out_sb)
        if c == 0 and it > 0:
            del iters[it - 1]
    del pend

    ctx.close()


def _build_nc():
    nc = bacc.Bacc(None, target_bir_lowering=False)
    q_d = nc.declare_dram_parameter("q", [L, HL, D], F32, isOutput=False)
    k_d = nc.declare_dram_parameter("k", [L, HL, D], F32, isOutput=False)
    v_d = nc.declare_dram_parameter("v", [L, HL, D], F32, isOutput=False)
    msk_d = nc.declare_dram_parameter("mask", [128, HL * 128], F16, isOutput=False)
    idn_d = nc.declare_dram_parameter("iden", [128, 128], F16, isOutput=False)
    out_d = nc.declare_dram_parameter("out", [L, HL, D], F32, isOutput=True)
    with tile.TileContext(nc) as tc:
        _build_body(nc, tc, q_d, k_d, v_d, msk_d, idn_d, out_d)
    nc.compile()
    return nc


def _get_nc():
    if "nc" not in _CACHE:
        _CACHE["nc"] = _build_nc()
    return _CACHE["nc"]


def _const_inputs():
    s = np.arange(128)[:, None]
    t = np.arange(128)[None, :]
    triu = (s <= t).astype(np.float16)
    mask = np.tile(triu, (1, HL))
    iden = np.eye(128, dtype=np.float16)
    return mask, iden


def run(inputs, trace=False, trace_cores=None):
    q = np.asarray(inputs["queries"])
    k = np.asarray(inputs["keys"])
    v = np.asarray(inputs["values"])
    mask, iden = _const_inputs()
    in_maps = []
    for c in range(8):
        n, hs = c // 2, (c % 2) * HL
        in_maps.append({
            "q": np.ascontiguousarray(q[n, :, hs : hs + HL, :]),
            "k": np.ascontiguousarray(k[n, :, hs : hs + HL, :]),
            "v": np.ascontiguousarray(v[n, :, hs : hs + HL, :]),
            "mask": mask,
            "iden": iden,
        })
    nc = _get_nc()
    res = run_bass_kernel_spmd(
        nc, in_maps, list(range(8)), trace=trace,
        trace_cores=trace_cores,
    )
    out = np.empty((N, L, H, D), np.float32)
    for c in range(8):
        n, hs = c // 2, (c % 2) * HL
        out[n, :, hs : hs + HL, :] = res.results[c]["out"]
    return out, res


def kernel(**inputs):
    out, _ = run(inputs)
    return out



# revision 20
# speedup vs baseline: 1.0086x; 1.0086x over previous
"""Causal linear attention (elu+1 feature map) Trainium2 Bass kernel.

Problem: queries/keys/values [N=4, L=8192, H=8, D=64] f32.
  Q = elu(q)+1, K = elu(k)+1
  Z = 1 / (sum_d Q[l,d] * cumsum_l(K)[l,d] + eps)
  out = causal_linear(Q, K, V) * Z          (chunked scan, CHUNK=128)

Sharding: 8 cores = (batch n in 0..3) x (head half in 0..1); each core
processes its [L, 4, 64] shard with no cross-core communication.

Per-core algorithm (per head h, chunk c of 128 rows):
  qT,kT = per-head PE transposes of the feature-mapped inputs, all at
          partitions 0-63 (uniform K=64 row group: consecutive matmuls
          with *different* partial row groups crash the PE here)
  attnT[s,t] = matmul(lhsT=kT_h, rhs=qT_h), masked by triu(s<=t)
  out[t,:]   = matmul(lhsT=attn_m, rhs=v'_h) + matmul(lhsT=qT_h, rhs=S'_h)
  v'    = v with a ones column appended, so the same matmuls also produce
          the normalizer denominator (rowsum(attn) + q . ksum)
  S'    = [S | ksum] PSUM accumulator over chunks (f32, exact)
  out_final = out * 1/denom via one broadcast tensor_tensor per chunk

Operands are fp16 (PE fast-weight-load + DVE 16-bit modes); all matmul
accumulation is f32 in PSUM; output f32.
"""

import numpy as np

import concourse.bass as bass_mod
import concourse.bacc as bacc
import concourse.tile as tile
from concourse import mybir
from concourse.bass_utils import run_bass_kernel_spmd

F16 = mybir.dt.float16
F32 = mybir.dt.float32
ALU = mybir.AluOpType
AF = mybir.ActivationFunctionType

N, L, H, D = 4, 8192, 8, 64
HL = 4                # heads per core
CHUNK = 128
NCHUNK = L // CHUNK   # 64
ITER_CHUNKS = 8       # chunks per outer iteration
NITER = NCHUNK // ITER_CHUNKS
CW = HL * D           # 256 floats per row in the shard

_CACHE = {}


def _build_body(nc, tc, q_d, k_d, v_d, msk_d, idn_d, out_d):
    qv = q_d.ap().rearrange("(i c p) h d -> i p c (h d)", p=CHUNK, c=ITER_CHUNKS)
    kv = k_d.ap().rearrange("(i c p) h d -> i p c (h d)", p=CHUNK, c=ITER_CHUNKS)
    vv = v_d.ap().rearrange("(i c p) h d -> i p c (h d)", p=CHUNK, c=ITER_CHUNKS)
    ov = out_d.ap().rearrange("(i c p) h d -> i p c (h d)", p=CHUNK, c=ITER_CHUNKS)
    IW = ITER_CHUNKS * CW        # 1024
    E1 = D + 1                   # 65: value cols + ones col
    VW = ITER_CHUNKS * HL * E1   # 1040: v' tile width

    def i3(t, lo, width):
        return t[:, lo : lo + width].rearrange("p (c w) -> p c w", w=CW)

    from contextlib import ExitStack
    ctx = ExitStack()
    consts = ctx.enter_context(tc.tile_pool(name="consts", bufs=1))
    mask_sb = consts.tile([128, HL * 128], F16)
    nc.sync.dma_start(out=mask_sb[:], in_=msk_d.ap())
    iden_sb = consts.tile([128, 128], F16)
    nc.sync.dma_start(out=iden_sb[:], in_=idn_d.ap())
    zero_sb = consts.tile([128, HL * E1], F16)
    nc.vector.memset(zero_sb[:], 0.0)
    ones_sb = consts.tile([128, 1], F16)
    nc.vector.memset(ones_sb[:], 1.0)

    qk_pool = ctx.enter_context(tc.tile_pool(name="qk", bufs=2))
    fm_pool = ctx.enter_context(tc.tile_pool(name="fm", bufs=2))
    v_pool = ctx.enter_context(tc.tile_pool(name="v", bufs=2))
    qkt_pool = ctx.enter_context(tc.tile_pool(name="qkt", bufs=3))
    am_pool = ctx.enter_context(tc.tile_pool(name="am", bufs=4))
    ssb_pool = ctx.enter_context(tc.tile_pool(name="ssb", bufs=3))
    rec_pool = ctx.enter_context(tc.tile_pool(name="rec", bufs=3))
    osb_pool = ctx.enter_context(tc.tile_pool(name="osb", bufs=2))

    ps_tp = ctx.enter_context(tc.tile_pool(name="ps_tp", bufs=2, space="PSUM"))
    ps_at = ctx.enter_context(tc.tile_pool(name="ps_at", bufs=3, space="PSUM"))
    ps_out = ctx.enter_context(tc.tile_pool(name="ps_out", bufs=2, space="PSUM"))
    ps_S = ctx.enter_context(tc.tile_pool(name="ps_S", bufs=1, space="PSUM"))

    # persistent state accumulator, all heads at partitions 0-63:
    # head h -> cols h*65..h*65+64 = S[d,e], col h*65+64 = ksum[d].
    # Opened by the first update (start=True); closed after chunk 0 by a
    # zero-accumulate matmul so later PSUM reads are legal; all later
    # updates accumulate with the group closed (skip_group_check).
    S_ps = ps_S.tile([64, HL * E1], F32)

    def load_iter(it):
        qk_t = qk_pool.tile([128, 2 * IW], F16)
        nc.gpsimd.dma_start(out=i3(qk_t, 0, IW), in_=qv[it])
        nc.gpsimd.dma_start(out=i3(qk_t, IW, IW), in_=kv[it])
        # v in straight (c h d) layout: one cast-DMA per iteration; the
        # denominator ones-column is handled by explicit 1-col matmuls
        v_t = v_pool.tile([128, IW], F16)
        nc.gpsimd.dma_start(out=i3(v_t, 0, IW), in_=vv[it])

        # feature map: elu(x)+1 == min(exp(x), relu(x)+1)
        e_t = fm_pool.tile([128, 2 * IW], F16, tag="fm_e")
        nc.scalar.activation(e_t[:], qk_t[:], AF.Exp)
        r_t = fm_pool.tile([128, 2 * IW], F16, tag="fm_r")
        nc.vector.tensor_scalar(r_t[:], qk_t[:], 0.0, 1.0, ALU.max, ALU.add)
        f_t = fm_pool.tile([128, 2 * IW], F16, tag="fm_f")
        nc.vector.tensor_tensor(f_t[:], e_t[:], r_t[:], ALU.min)

        out_sb = osb_pool.tile([128, IW], F16)
        return f_t, v_t, out_sb

    def phase_a1(g, f_t):
        """transposes + qkT PSUM->SBUF copies for chunk g (no S dependency)"""
        c = g % ITER_CHUNKS
        fq = f_t[:, c * CW : (c + 1) * CW]
        fk = f_t[:, IW + c * CW : IW + (c + 1) * CW]
        tp = ps_tp.tile([64, 8 * 128], F16)
        for h in range(HL):
            nc.tensor.transpose(
                tp[:, h * 128 : (h + 1) * 128],
                fq[:, h * D : (h + 1) * D], iden_sb[:])
            nc.tensor.transpose(
                tp[:, 512 + h * 128 : 512 + (h + 1) * 128],
                fk[:, h * D : (h + 1) * D], iden_sb[:])
        qkT = qkt_pool.tile([64, 8 * 128], F16)
        nc.scalar.copy(qkT[:, 0:768], tp[:, 0:768])
        nc.vector.tensor_copy(qkT[:, 768:1024], tp[:, 768:1024])
        return qkT, fk

    def phase_a2(qkT):
        """attn matmuls + mask for chunk g"""
        at_ps = ps_at.tile([128, HL * 128], F32)
        for h in range(HL):
            nc.tensor.matmul(
                at_ps[:, h * 128 : (h + 1) * 128],
                lhsT=qkT[:, 512 + h * 128 : 512 + (h + 1) * 128],
                rhs=qkT[:, h * 128 : (h + 1) * 128],
                start=(h == 0), stop=(h == HL - 1),
            )
        am = am_pool.tile([128, HL * 128], F16)
        nc.vector.tensor_tensor(am[:], at_ps[:], mask_sb[:], ALU.mult)
        return am

    def phase_b(g, am, qkT, fk, v_t, out_sb):
        """S copy, out matmuls, normalize, state update for chunk g"""
        c = g % ITER_CHUNKS
        S_sb = None
        if g > 0:
            S_sb = ssb_pool.tile([64, HL * E1], F16)
            nc.scalar.copy(S_sb[:], S_ps[:])

        # per head: am@v (values), am@ones (denom rowsum), q@S' (recurrent)
        out_ps = ps_out.tile([128, HL * E1], F32)
        n_mm = HL * (2 if g == 0 else 3)
        mm_i = 0
        for h in range(HL):
            co = h * E1
            amh = am[:, h * 128 : (h + 1) * 128]
            nc.tensor.matmul(
                out_ps[:, co : co + D],
                lhsT=amh,
                rhs=v_t[:, (c * HL + h) * D : (c * HL + h + 1) * D],
                start=(mm_i == 0), stop=(mm_i == n_mm - 1),
            )
            mm_i += 1
            nc.tensor.matmul(
                out_ps[:, co + D : co + E1],
                lhsT=amh,
                rhs=ones_sb[:],
                start=False, stop=(mm_i == n_mm - 1),
            )
            mm_i += 1
            if g > 0:
                nc.tensor.matmul(
                    out_ps[:, co : co + E1],
                    lhsT=qkT[:, h * 128 : (h + 1) * 128],
                    rhs=S_sb[:, co : co + E1],
                    start=False, stop=(mm_i == n_mm - 1),
                )
                mm_i += 1

        # ---- normalize: out = out_ps[:, value cols] * (1/denom)
        rec = rec_pool.tile([128, HL], F32)
        opv = out_ps[:].rearrange("p (h e) -> p h e", h=HL)
        nc.vector.reciprocal(rec[:], opv[:, :, D])
        rbc = bass_mod.AP(tensor=rec.tensor, offset=rec[:].offset,
                          ap=[list(rec[:].ap[0]), [1, HL], [0, D]])
        nc.vector.tensor_tensor(
            out_sb[:, c * CW : (c + 1) * CW].rearrange(
                "p (h e) -> p h e", h=HL),
            opv[:, :, 0:D], rbc, ALU.mult)

        # ---- state update (skip after last chunk)
        if g < NCHUNK - 1:
            for h in range(HL):
                fkh = fk[:, h * D : (h + 1) * D]
                nc.tensor.matmul(
                    S_ps[:, h * E1 : h * E1 + D],
                    lhsT=fkh,
                    rhs=v_t[:, (c * HL + h) * D : (c * HL + h + 1) * D],
                    start=(g == 0 and h == 0), stop=False,
                    skip_group_check=(g > 0),
                )
                nc.tensor.matmul(
                    S_ps[:, h * E1 + D : h * E1 + E1],
                    lhsT=fkh,
                    rhs=ones_sb[:],
                    start=False, stop=False,
                    skip_group_check=(g > 0),
                )
            if g == 0:
                # close the accumulation group (adds zeros) so later
                # engine reads of S_ps pass the sim's group check
                nc.tensor.matmul(
                    S_ps[:], lhsT=iden_sb[:, 0:64], rhs=zero_sb[:, 0 : HL * E1],
                    start=False, stop=True)

        if c == ITER_CHUNKS - 1:
            it = g // ITER_CHUNKS
            nc.gpsimd.dma_start(
                out=ov[it], in_=out_sb[:].rearrange("p (c w) -> p c w", w=CW))

    # software-pipelined PE issue order per step:
    #   [transposes(g)] [out/state(g-1)] [attn(g)]
    # so the Act qkT-copy of chunk g overlaps out/state of g-1, and the DVE
    # mask of chunk g overlaps the transposes of g+1
    pend = None
    iters = {}
    for g in range(NCHUNK):
        it, c = divmod(g, ITER_CHUNKS)
        if c == 0:
            iters[it] = load_iter(it)
        f_t, v_t, out_sb = iters[it]
        qkT, fk = phase_a1(g, f_t)
        am = phase_a2(qkT)
        phase_b(g, am, qkT, fk, v_t, out_sb)
        if c == 0 and it > 0:
            del iters[it - 1]
    del pend

    ctx.close()


def _build_nc():
    nc = bacc.Bacc(None, target_bir_lowering=False)
    q_d = nc.declare_dram_parameter("q", [L, HL, D], F32, isOutput=False)
    k_d = nc.declare_dram_parameter("k", [L, HL, D], F32, isOutput=False)
    v_d = nc.declare_dram_parameter("v", [L, HL, D], F32, isOutput=False)
    msk_d = nc.declare_dram_parameter("mask", [128, HL * 128], F16, isOutput=False)
    idn_d = nc.declare_dram_parameter("iden", [128, 128], F16, isOutput=False)
    out_d = nc.declare_dram_parameter("out", [L, HL, D], F32, isOutput=True)
    with tile.TileContext(nc) as tc:
        _build_body(nc, tc, q_d, k_d, v_d, msk_d, idn_d, out_d)
    nc.compile()
    return nc


def _get_nc():
    if "nc" not in _CACHE:
        _CACHE["nc"] = _build_nc()
    return _CACHE["nc"]


def _const_inputs():
    s = np.arange(128)[:, None]
    t = np.arange(128)[None, :]
    triu = (s <= t).astype(np.float16)
    mask = np.tile(triu, (1, HL))
    iden = np.eye(128, dtype=np.float16)
    return mask, iden


def run(inputs, trace=False, trace_cores=None):
    q = np.asarray(inputs["queries"])
    k = np.asarray(inputs["keys"])
    v = np.asarray(inputs["values"])
    mask, iden = _const_inputs()
    in_maps = []
    for c in range(8):
        n, hs = c // 2, (c % 2) * HL
        in_maps.append({
            "q": np.ascontiguousarray(q[n, :, hs : hs + HL, :]),
            "k": np.ascontiguousarray(k[n, :, hs : hs + HL, :]),
            "v": np.ascontiguousarray(v[n, :, hs : hs + HL, :]),
            "mask": mask,
            "iden": iden,
        })
    nc = _get_nc()
    res = run_bass_kernel_spmd(
        nc, in_maps, list(range(8)), trace=trace,
        trace_cores=trace_cores,
    )
    out = np.empty((N, L, H, D), np.float32)
    for c in range(8):
        n, hs = c // 2, (c % 2) * HL
        out[n, :, hs : hs + HL, :] = res.results[c]["out"]
    return out, res


def kernel(**inputs):
    out, _ = run(inputs)
    return out



# revision 25
# speedup vs baseline: 1.0519x; 1.0430x over previous
"""Causal linear attention (elu+1 feature map) Trainium2 Bass kernel.

Problem: queries/keys/values [N=4, L=8192, H=8, D=64] f32.
  Q = elu(q)+1, K = elu(k)+1
  Z = 1 / (sum_d Q[l,d] * cumsum_l(K)[l,d] + eps)
  out = causal_linear(Q, K, V) * Z          (chunked scan, CHUNK=128)

Sharding: 8 cores = (batch n in 0..3) x (head half in 0..1); each core
processes its [L, 4, 64] shard with no cross-core communication.

Per-core algorithm (per head h, chunk c of 128 rows):
  qT,kT = per-head PE transposes of the feature-mapped inputs, all at
          partitions 0-63 (uniform K=64 row group: consecutive matmuls
          with *different* partial row groups crash the PE here)
  attnT[s,t] = matmul(lhsT=kT_h, rhs=qT_h), masked by triu(s<=t)
  out[t,:]   = matmul(lhsT=attn_m, rhs=v'_h) + matmul(lhsT=qT_h, rhs=S'_h)
  v'    = v with a ones column appended, so the same matmuls also produce
          the normalizer denominator (rowsum(attn) + q . ksum)
  S'    = [S | ksum] PSUM accumulator over chunks (f32, exact)
  out_final = out * 1/denom via one broadcast tensor_tensor per chunk

Operands are fp16 (PE fast-weight-load + DVE 16-bit modes); all matmul
accumulation is f32 in PSUM; output f32.
"""

import numpy as np

import concourse.bass as bass_mod
import concourse.bacc as bacc
import concourse.tile as tile
from concourse import mybir
from concourse.bass_utils import run_bass_kernel_spmd

F16 = mybir.dt.float16
F32 = mybir.dt.float32
ALU = mybir.AluOpType
AF = mybir.ActivationFunctionType

N, L, H, D = 4, 8192, 8, 64
HL = 4                # heads per core
CHUNK = 128
NCHUNK = L // CHUNK   # 64
ITER_CHUNKS = 8       # chunks per outer iteration
NITER = NCHUNK // ITER_CHUNKS
CW = HL * D           # 256 floats per row in the shard

_CACHE = {}


def _build_body(nc, tc, q_d, k_d, v_d, msk_d, idn_d, out_d):
    qv = q_d.ap().rearrange("(i c p) h d -> i p c (h d)", p=CHUNK, c=ITER_CHUNKS)
    kv = k_d.ap().rearrange("(i c p) h d -> i p c (h d)", p=CHUNK, c=ITER_CHUNKS)
    vv = v_d.ap().rearrange("(i c p) h d -> i p c (h d)", p=CHUNK, c=ITER_CHUNKS)
    ov = out_d.ap().rearrange("(i c p) h d -> i p c (h d)", p=CHUNK, c=ITER_CHUNKS)
    IW = ITER_CHUNKS * CW        # 1024
    E1 = D + 1                   # 65: value cols + ones col
    VW = ITER_CHUNKS * HL * E1   # 1040: v' tile width

    def i3(t, lo, width):
        return t[:, lo : lo + width].rearrange("p (c w) -> p c w", w=CW)

    from contextlib import ExitStack
    ctx = ExitStack()
    consts = ctx.enter_context(tc.tile_pool(name="consts", bufs=1))
    mask_sb = consts.tile([128, HL * 128], F16)
    nc.sync.dma_start(out=mask_sb[:], in_=msk_d.ap())
    iden_sb = consts.tile([128, 128], F16)
    nc.sync.dma_start(out=iden_sb[:], in_=idn_d.ap())
    zero_sb = consts.tile([128, HL * E1], F16)
    nc.vector.memset(zero_sb[:], 0.0)
    ones_sb = consts.tile([128, 1], F16)
    nc.vector.memset(ones_sb[:], 1.0)

    qk_pool = ctx.enter_context(tc.tile_pool(name="qk", bufs=2))
    fm_pool = ctx.enter_context(tc.tile_pool(name="fm", bufs=2))
    v_pool = ctx.enter_context(tc.tile_pool(name="v", bufs=2))
    qkt_pool = ctx.enter_context(tc.tile_pool(name="qkt", bufs=4))
    am_pool = ctx.enter_context(tc.tile_pool(name="am", bufs=6))
    ssb_pool = ctx.enter_context(tc.tile_pool(name="ssb", bufs=4))
    rec_pool = ctx.enter_context(tc.tile_pool(name="rec", bufs=4))
    osb_pool = ctx.enter_context(tc.tile_pool(name="osb", bufs=2))

    ps_tp = ctx.enter_context(tc.tile_pool(name="ps_tp", bufs=2, space="PSUM"))
    ps_at = ctx.enter_context(tc.tile_pool(name="ps_at", bufs=3, space="PSUM"))
    ps_out = ctx.enter_context(tc.tile_pool(name="ps_out", bufs=2, space="PSUM"))
    ps_S = ctx.enter_context(tc.tile_pool(name="ps_S", bufs=1, space="PSUM"))

    # persistent state accumulator, all heads at partitions 0-63:
    # head h -> cols h*65..h*65+64 = S[d,e], col h*65+64 = ksum[d].
    # Opened by the first update (start=True); closed after chunk 0 by a
    # zero-accumulate matmul so later PSUM reads are legal; all later
    # updates accumulate with the group closed (skip_group_check).
    S_ps = ps_S.tile([64, HL * E1], F32)

    def load_iter(it):
        qk_t = qk_pool.tile([128, 2 * IW], F16)
        nc.gpsimd.dma_start(out=i3(qk_t, 0, IW), in_=qv[it])
        nc.gpsimd.dma_start(out=i3(qk_t, IW, IW), in_=kv[it])
        # v in straight (c h d) layout: one cast-DMA per iteration; the
        # denominator ones-column is handled by explicit 1-col matmuls
        v_t = v_pool.tile([128, IW], F16)
        nc.gpsimd.dma_start(out=i3(v_t, 0, IW), in_=vv[it])

        # feature map: elu(x)+1 == min(exp(x), relu(x)+1)
        e_t = fm_pool.tile([128, 2 * IW], F16, tag="fm_e")
        nc.scalar.activation(e_t[:], qk_t[:], AF.Exp)
        r_t = fm_pool.tile([128, 2 * IW], F16, tag="fm_r")
        nc.vector.tensor_scalar(r_t[:], qk_t[:], 0.0, 1.0, ALU.max, ALU.add)
        f_t = fm_pool.tile([128, 2 * IW], F16, tag="fm_f")
        nc.vector.tensor_tensor(f_t[:], e_t[:], r_t[:], ALU.min)

        out_sb = osb_pool.tile([128, IW], F16)
        return f_t, v_t, out_sb

    def phase_a1(g, f_t):
        """transposes + qkT PSUM->SBUF copies for chunk g (no S dependency)"""
        c = g % ITER_CHUNKS
        fq = f_t[:, c * CW : (c + 1) * CW]
        fk = f_t[:, IW + c * CW : IW + (c + 1) * CW]
        tp = ps_tp.tile([64, 8 * 128], F16)
        for h in range(HL):
            nc.tensor.transpose(
                tp[:, h * 128 : (h + 1) * 128],
                fq[:, h * D : (h + 1) * D], iden_sb[:])
            nc.tensor.transpose(
                tp[:, 512 + h * 128 : 512 + (h + 1) * 128],
                fk[:, h * D : (h + 1) * D], iden_sb[:])
        qkT = qkt_pool.tile([64, 8 * 128], F16)
        nc.scalar.copy(qkT[:, 0:768], tp[:, 0:768])
        nc.vector.tensor_copy(qkT[:, 768:1024], tp[:, 768:1024])
        return qkT, fk

    def phase_a2(qkT):
        """attn matmuls + mask for chunk g"""
        at_ps = ps_at.tile([128, HL * 128], F32)
        for h in range(HL):
            nc.tensor.matmul(
                at_ps[:, h * 128 : (h + 1) * 128],
                lhsT=qkT[:, 512 + h * 128 : 512 + (h + 1) * 128],
                rhs=qkT[:, h * 128 : (h + 1) * 128],
                start=(h == 0), stop=(h == HL - 1),
            )
        am = am_pool.tile([128, HL * 128], F16)
        nc.vector.tensor_tensor(am[:], at_ps[:], mask_sb[:], ALU.mult)
        return am

    def phase_b(g, am, qkT, fk, v_t, out_sb, S_sb):
        """out matmuls, state update, next-chunk S copy, normalize for chunk g.

        S_sb (state after chunk g-1) is copied at the END of phase_b(g-1) so
        the Act copy overlaps all of phase A instead of gating the out mms."""
        c = g % ITER_CHUNKS

        # per head: am@v (values), am@ones (denom rowsum), q@S' (recurrent)
        out_ps = ps_out.tile([128, HL * E1], F32)
        n_mm = HL * (2 if g == 0 else 3)
        mm_i = 0
        for h in range(HL):
            co = h * E1
            amh = am[:, h * 128 : (h + 1) * 128]
            nc.tensor.matmul(
                out_ps[:, co : co + D],
                lhsT=amh,
                rhs=v_t[:, (c * HL + h) * D : (c * HL + h + 1) * D],
                start=(mm_i == 0), stop=(mm_i == n_mm - 1),
            )
            mm_i += 1
            nc.tensor.matmul(
                out_ps[:, co + D : co + E1],
                lhsT=amh,
                rhs=ones_sb[:],
                start=False, stop=(mm_i == n_mm - 1),
            )
            mm_i += 1
            if g > 0:
                nc.tensor.matmul(
                    out_ps[:, co : co + E1],
                    lhsT=qkT[:, h * 128 : (h + 1) * 128],
                    rhs=S_sb[:, co : co + E1],
                    start=False, stop=(mm_i == n_mm - 1),
                )
                mm_i += 1

        # ---- normalize: out = out_ps[:, value cols] * (1/denom)
        rec = rec_pool.tile([128, HL], F32)
        opv = out_ps[:].rearrange("p (h e) -> p h e", h=HL)
        nc.vector.reciprocal(rec[:], opv[:, :, D])
        rbc = bass_mod.AP(tensor=rec.tensor, offset=rec[:].offset,
                          ap=[list(rec[:].ap[0]), [1, HL], [0, D]])
        nc.vector.tensor_tensor(
            out_sb[:, c * CW : (c + 1) * CW].rearrange(
                "p (h e) -> p h e", h=HL),
            opv[:, :, 0:D], rbc, ALU.mult)

        # ---- state update (skip after last chunk)
        if g < NCHUNK - 1:
            for h in range(HL):
                fkh = fk[:, h * D : (h + 1) * D]
                nc.tensor.matmul(
                    S_ps[:, h * E1 : h * E1 + D],
                    lhsT=fkh,
                    rhs=v_t[:, (c * HL + h) * D : (c * HL + h + 1) * D],
                    start=(g == 0 and h == 0), stop=False,
                    skip_group_check=(g > 0),
                )
                nc.tensor.matmul(
                    S_ps[:, h * E1 + D : h * E1 + E1],
                    lhsT=fkh,
                    rhs=ones_sb[:],
                    start=False, stop=False,
                    skip_group_check=(g > 0),
                )
            if g == 0:
                # close the accumulation group (adds zeros) so later
                # engine reads of S_ps pass the sim's group check
                nc.tensor.matmul(
                    S_ps[:], lhsT=iden_sb[:, 0:64], rhs=zero_sb[:, 0 : HL * E1],
                    start=False, stop=True)

        S_sb_next = None
        if g < NCHUNK - 1:
            S_sb_next = ssb_pool.tile([64, HL * E1], F16)
            nc.scalar.copy(S_sb_next[:], S_ps[:])

        if c == ITER_CHUNKS - 1:
            it = g // ITER_CHUNKS
            nc.gpsimd.dma_start(
                out=ov[it], in_=out_sb[:].rearrange("p (c w) -> p c w", w=CW))
        return S_sb_next

    # software-pipelined PE issue order per step:
    #   [transposes(g)] [out/state(g-1)] [attn(g)]
    # so the Act qkT-copy of chunk g overlaps out/state of g-1, and the DVE
    # mask of chunk g overlaps the transposes of g+1
    iters = {}
    S_sb = None
    for g in range(NCHUNK):
        it, c = divmod(g, ITER_CHUNKS)
        if c == 0:
            iters[it] = load_iter(it)
        f_t, v_t, out_sb = iters[it]
        qkT, fk = phase_a1(g, f_t)
        am = phase_a2(qkT)
        S_sb = phase_b(g, am, qkT, fk, v_t, out_sb, S_sb)
        if c == 0 and it > 0:
            del iters[it - 1]

    ctx.close()


def _build_nc():
    nc = bacc.Bacc(None, target_bir_lowering=False)
    q_d = nc.declare_dram_parameter("q", [L, HL, D], F32, isOutput=False)
    k_d = nc.declare_dram_parameter("k", [L, HL, D], F32, isOutput=False)
    v_d = nc.declare_dram_parameter("v", [L, HL, D], F32, isOutput=False)
    msk_d = nc.declare_dram_parameter("mask", [128, HL * 128], F16, isOutput=False)
    idn_d = nc.declare_dram_parameter("iden", [128, 128], F16, isOutput=False)
    out_d = nc.declare_dram_parameter("out", [L, HL, D], F32, isOutput=True)
    with tile.TileContext(nc) as tc:
        _build_body(nc, tc, q_d, k_d, v_d, msk_d, idn_d, out_d)
    nc.compile()
    return nc


def _get_nc():
    if "nc" not in _CACHE:
        _CACHE["nc"] = _build_nc()
    return _CACHE["nc"]


def _const_inputs():
    s = np.arange(128)[:, None]
    t = np.arange(128)[None, :]
    triu = (s <= t).astype(np.float16)
    mask = np.tile(triu, (1, HL))
    iden = np.eye(128, dtype=np.float16)
    return mask, iden


def run(inputs, trace=False, trace_cores=None):
    q = np.asarray(inputs["queries"])
    k = np.asarray(inputs["keys"])
    v = np.asarray(inputs["values"])
    mask, iden = _const_inputs()
    in_maps = []
    for c in range(8):
        n, hs = c // 2, (c % 2) * HL
        in_maps.append({
            "q": np.ascontiguousarray(q[n, :, hs : hs + HL, :]),
            "k": np.ascontiguousarray(k[n, :, hs : hs + HL, :]),
            "v": np.ascontiguousarray(v[n, :, hs : hs + HL, :]),
            "mask": mask,
            "iden": iden,
        })
    nc = _get_nc()
    res = run_bass_kernel_spmd(
        nc, in_maps, list(range(8)), trace=trace,
        trace_cores=trace_cores,
    )
    out = np.empty((N, L, H, D), np.float32)
    for c in range(8):
        n, hs = c // 2, (c % 2) * HL
        out[n, :, hs : hs + HL, :] = res.results[c]["out"]
    return out, res


def kernel(**inputs):
    out, _ = run(inputs)
    return out

